# revision 1
# baseline (speedup 1.0000x reference)
"""Trainium2 Bass kernel for banded local attention (kernel_size=128).

Problem: x[4,4096,512]; q = x@Wq.T+bq, k = x@Wk.T+bk (H=512);
scores = q@k.T masked to |i-j|<128; softmax; out = attn @ x.

Sharding: 8 cores = 4 batches x 2 sequence halves (2048 queries each) with a
128-row halo of keys on each side (2304 local key rows, zero padded at the
global sequence edges). For the h=1 half the sequence is passed REVERSED so
the padded/invalid key region is always local rows [0,128) and the edge mask
is only needed for query block 0 -> all 8 cores run the identical program
(pure SPMD, no collectives). Host un-reverses the h=1 outputs.

Per-core data layout (all fp32, matmuls in float32r = full-rate fp22):
  xT   [512, 2304]  x_halo transposed (d on partitions) - rhs/lhsT for projs
  xrow [2304, 512]  x_halo row-major - rhs ("values") for attn @ x
  wqT/wkT [512,512] weight transposed [d, h] - lhsT for projections
  bq/bk [512]       biases (added via ACT Identity during PSUM->SBUF copy)
  masks [2,128,384] additive band masks (0 / -1e30); slot 1 = edge variant
On chip:
  qT [h,2048] = wqT.T @ xT (+bq)  4 h-tiles; serves as lhsT for scores
  kT [h,2304] = wkT.T @ xT (+bk)  4 h-tiles; serves as rhs for scores
  per 128-query block qb: s[128,384] = qT_blk.T @ kT_window (PSUM),
  s += mask (DVE), rowmax m (DVE), p = exp(s - m) with accumulated row
  sum l (ACT), pT = PE-transpose(p), out = pT.T @ xrow_window, scaled by
  1/l during the PSUM->SBUF copy (ACT, scale AP).
"""
import sys

if "/opt/trn_rl_repo" not in sys.path:
    sys.path.insert(0, "/opt/trn_rl_repo")

import numpy as np

B, S, D, H = 4, 4096, 512, 512
KS = 128
HALF = S // 2            # 2048 queries per core
HALO = KS                # 128
SK = HALF + 2 * HALO     # 2304 local key rows
WIN = 3 * 128            # 384-wide key window per query block
NBLK = HALF // 128       # 16 query blocks
NEG = -1e30
N_CORES = 8

F32 = None  # set after import
_cached = {}


def _build_program():
    import concourse.bass as bass
    import concourse.tile as tile
    import concourse.mybir as mybir
    from concourse import bacc

    f32 = mybir.dt.float32
    f32r = mybir.dt.float32r
    AF = mybir.ActivationFunctionType
    AX = mybir.AxisListType

    nc = bacc.Bacc("TRN2", target_bir_lowering=False, debug=False,
                   num_devices=N_CORES)

    xT_d = nc.dram_tensor("xT", [D, SK], f32r, kind="ExternalInput").ap()
    xrow_d = nc.dram_tensor("xrow", [SK, D], f32r, kind="ExternalInput").ap()
    wqT_d = nc.dram_tensor("wqT", [D, H], f32r, kind="ExternalInput").ap()
    wkT_d = nc.dram_tensor("wkT", [D, H], f32r, kind="ExternalInput").ap()
    bq_d = nc.dram_tensor("bq", [H, 1], f32, kind="ExternalInput").ap()
    bk_d = nc.dram_tensor("bk", [H, 1], f32, kind="ExternalInput").ap()
    masks_d = nc.dram_tensor("masks", [2, 128, WIN], f32,
                             kind="ExternalInput").ap()
    out_d = nc.dram_tensor("out", [HALF, D], f32, kind="ExternalOutput").ap()

    DT = D // 128   # 4 d-tiles
    HT = H // 128   # 4 h-tiles
    JT = SK // 128  # 18 key row tiles

    with tile.TileContext(nc) as tc:
        with (
            tc.tile_pool(name="big", bufs=1) as big,
            tc.tile_pool(name="work", bufs=3) as work,
            tc.tile_pool(name="stat", bufs=4) as stat,
            tc.tile_pool(name="psA", bufs=2, space="PSUM") as psA,
            tc.tile_pool(name="psB", bufs=2, space="PSUM") as psB,
        ):
            # ---- resident inputs ----
            wq = [big.tile([128, H], f32r, tag=f"wq{t}", name=f"wq{t}") for t in range(DT)]
            wk = [big.tile([128, H], f32r, tag=f"wk{t}", name=f"wk{t}") for t in range(DT)]
            for t in range(DT):
                nc.sync.dma_start(wq[t], wqT_d[t * 128:(t + 1) * 128, :])
                nc.sync.dma_start(wk[t], wkT_d[t * 128:(t + 1) * 128, :])
            bq = [big.tile([128, 1], f32, tag=f"bq{t}", name=f"bq{t}") for t in range(HT)]
            bk = [big.tile([128, 1], f32, tag=f"bk{t}", name=f"bk{t}") for t in range(HT)]
            for t in range(HT):
                nc.sync.dma_start(bq[t], bq_d[t * 128:(t + 1) * 128, :])
                nc.sync.dma_start(bk[t], bk_d[t * 128:(t + 1) * 128, :])
            masks = [big.tile([128, WIN], f32, tag=f"mask{i}", name=f"maskt{i}") for i in range(2)]
            for i in range(2):
                nc.sync.dma_start(masks[i], masks_d[i])
            xT = [big.tile([128, SK], f32r, tag=f"xT{t}", name=f"xT{t}") for t in range(DT)]
            for t in range(DT):
                nc.sync.dma_start(xT[t], xT_d[t * 128:(t + 1) * 128, :])

            # ---- projections: qT[h,i] and kT[h,j] ----
            qT = [big.tile([128, HALF], f32r, tag=f"qT{t}", name=f"qT{t}") for t in range(HT)]
            kT = [big.tile([128, SK], f32r, tag=f"kT{t}", name=f"kT{t}") for t in range(HT)]

            def project(dst, w_tiles, bias, n_cols, off=0, on_dve=False):
                # dst[ht][h, c] = sum_d w[d, h] * xT[d, c] + bias[h]
                chunks = []
                c0 = 0
                while c0 < n_cols:
                    cw = min(512, n_cols - c0)
                    chunks.append((c0, cw))
                    c0 += cw
                for ht in range(HT):
                    for (c0, cw) in chunks:
                        ps = psA.tile([128, 512], f32, tag="proj")
                        for dt_i in range(DT):
                            nc.tensor.matmul(
                                ps[:, :cw],
                                lhsT=w_tiles[dt_i][:, ht * 128:(ht + 1) * 128],
                                rhs=xT[dt_i][:, off + c0:off + c0 + cw],
                                start=(dt_i == 0),
                                stop=(dt_i == DT - 1),
                            )
                        if on_dve:
                            nc.vector.tensor_scalar_add(
                                dst[ht][:, c0:c0 + cw], ps[:, :cw], bias[ht])
                        else:
                            nc.scalar.activation(
                                dst[ht][:, c0:c0 + cw], ps[:, :cw],
                                AF.Identity, bias=bias[ht], scale=1.0,
                            )

            project(qT, wq, bq, HALF, off=HALO, on_dve=True)
            project(kT, wk, bk, SK)

            # xrow ("values") DMAs issued after the projection instructions so
            # the Sync queue services weights/xT first; these 4.7MB stream in
            # while the PE is busy with the projections.
            xrow = [big.tile([128, D], f32r, tag=f"xr{j}", name=f"xr{j}")
                    for j in range(JT)]
            for j in range(JT):
                nc.sync.dma_start(xrow[j], xrow_d[j * 128:(j + 1) * 128, :])

            # ---- identity for PE transpose ----
            ident = big.tile([128, 128], f32, tag="ident")
            from concourse.masks import make_identity
            make_identity(nc, ident)

            # ---- per query-block attention ----
            for qb in range(NBLK):
                j0 = qb * 128  # key window start (local row / kT col)
                mask = masks[1] if qb == 0 else masks[0]

                s_ps = psA.tile([128, WIN], f32, tag="s")
                for ht in range(HT):
                    nc.tensor.matmul(
                        s_ps,
                        lhsT=qT[ht][:, qb * 128:(qb + 1) * 128],
                        rhs=kT[ht][:, j0:j0 + WIN],
                        start=(ht == 0),
                        stop=(ht == HT - 1),
                    )
                # s += mask ; rowmax ; p = exp(s - m), l = rowsum(p)
                s_sb = work.tile([128, WIN], f32, tag="s_sb")
                nc.vector.tensor_add(s_sb, s_ps, mask)
                m = stat.tile([128, 1], f32, tag="m")
                nc.vector.reduce_max(m, s_sb, axis=AX.X)
                negm = stat.tile([128, 1], f32, tag="negm")
                nc.scalar.mul(negm, m, -1.0)
                p_sb = work.tile([128, WIN], f32, tag="p_sb")
                lsum = stat.tile([128, 1], f32, tag="lsum")
                nc.scalar.activation(p_sb, s_sb, AF.Exp, bias=negm,
                                     scale=1.0, accum_out=lsum)
                rinv = stat.tile([128, 1], f32, tag="rinv")
                nc.vector.reciprocal(rinv, lsum)

                # transpose p -> pT (3 x [128,128])
                pT_ps = psB.tile([128, 3, 128], f32, tag="pT")
                for jt in range(3):
                    nc.tensor.transpose(
                        pT_ps[:, jt, :],
                        p_sb[:, jt * 128:(jt + 1) * 128],
                        ident,
                    )
                pT_sb = work.tile([128, 3, 128], f32r, tag="pT_sb")
                nc.vector.tensor_copy(pT_sb, pT_ps)

                # out_blk[i, d] = sum_j p[i, j] * xrow[j, d], scaled by 1/l
                o_ps = psB.tile([128, D], f32, tag="o")
                for jt in range(3):
                    nc.tensor.matmul(
                        o_ps,
                        lhsT=pT_sb[:, jt, :],
                        rhs=xrow[qb + jt],
                        start=(jt == 0),
                        stop=(jt == 2),
                    )
                o_sb = work.tile([128, D], f32, tag="o_sb")
                nc.scalar.activation(o_sb, o_ps, AF.Identity,
                                     bias=0.0, scale=rinv)
                nc.sync.dma_start(out_d[qb * 128:(qb + 1) * 128, :], o_sb)

    nc.compile()
    return nc


def _get_program():
    if "nc" not in _cached:
        _cached["nc"] = _build_program()
    return _cached["nc"]


def _make_masks():
    a = np.arange(128)[:, None]
    y = np.arange(WIN)[None, :]
    band = (y - a >= 1) & (y - a <= 255)
    base = np.where(band, 0.0, NEG).astype(np.float32)
    edge = np.where(band & (y >= 128), 0.0, NEG).astype(np.float32)
    return np.stack([base, edge])


def kernel(x, Wq_w, Wq_b, Wk_w, Wk_b, _trace=False):
    from concourse.bass_utils import run_bass_kernel_spmd

    x = np.ascontiguousarray(np.asarray(x, np.float32))
    wqT = np.ascontiguousarray(np.asarray(Wq_w, np.float32).T)
    wkT = np.ascontiguousarray(np.asarray(Wk_w, np.float32).T)
    bq = np.ascontiguousarray(np.asarray(Wq_b, np.float32))
    bk = np.ascontiguousarray(np.asarray(Wk_b, np.float32))
    masks = _make_masks()

    nc = _get_program()

    in_maps = []
    for core in range(N_CORES):
        b, h = divmod(core, 2)
        x_halo = np.zeros((SK, D), np.float32)
        if h == 0:
            x_halo[HALO:] = x[b, 0:HALF + HALO]
        else:
            x_halo[HALO:] = x[b, S - HALF - HALO:][::-1]
        in_maps.append({
            "xT": np.ascontiguousarray(x_halo.T),
            "xrow": x_halo,
            "wqT": wqT,
            "wkT": wkT,
            "bq": bq.reshape(H, 1),
            "bk": bk.reshape(H, 1),
            "masks": masks,
        })

    res = run_bass_kernel_spmd(nc, in_maps, core_ids=list(range(N_CORES)),
                               trace=_trace)
    _cached["last_result"] = res

    y = np.zeros((B, S, D), np.float32)
    for core in range(N_CORES):
        b, h = divmod(core, 2)
        o = res.results[core]["out"]
        if h == 0:
            y[b, :HALF] = o
        else:
            y[b, HALF:] = o[::-1]
    return y



# revision 14
# speedup vs baseline: 1.4051x; 1.4051x over previous
"""Trainium2 Bass kernel for banded local attention (kernel_size=128).

Problem: x[4,4096,512]; q = x@Wq.T+bq, k = x@Wk.T+bk (H=512);
scores = q@k.T masked to |i-j|<128; softmax; out = attn @ x.

Algebraic restructure: softmax is shift-invariant per row, so terms of
q_i.k_j constant in j drop out:
    q_i . k_j  ~  (x_i (Wq^T Wk) + bq^T Wk) . x_j = t_i . x_j
with A = Wq^T @ Wk [D,D] and wbeta = Wk^T @ bq folded on the host.
This removes the whole k projection (and its bias) from the device:
one projection t = x@A + wbeta, then s = t @ x^T over a 384-wide
sliding window, softmax, out = p @ x.

Softmax path: row max over the RAW (unmasked) 384 window (extra terms
only shift the max; shift cancels in p/l), p = exp(s - m) in bf16, the
band mask applied multiplicatively by a fused DVE tensor_tensor_reduce
that also emits the masked row sum l. The output is left UNnormalized
on device (o = pm @ x, plus l shipped separately); the host divides.

Sharding: 8 cores = 4 batches x 2 sequence halves (2048 queries each)
with 128-row key halos (2304 local rows, zero padded at the global
edges). The h=1 half is passed REVERSED so the padded region is always
local rows [0,128) -> all 8 cores run the identical program (pure
SPMD, no collectives). Host un-reverses the h=1 outputs.

Schedule: xT arrives in column chunks; the t projection is emitted in
4 x 512-column chunks interleaved with the 16 attention blocks
(2-deep software pipeline) so the PE never drains. PSUM: 3x[128,512]
shared proj/out pool + 3x[128,384] scores + 2x[128,3,128] transpose
= 8 banks.
"""
import sys

if "/opt/trn_rl_repo" not in sys.path:
    sys.path.insert(0, "/opt/trn_rl_repo")

import numpy as np

B, S, D, H = 4, 4096, 512, 512
KS = 128
HALF = S // 2            # 2048 queries per core
HALO = KS                # 128
SK = HALF + 2 * HALO     # 2304 local key rows
WIN = 3 * 128            # 384-wide key window per query block
NBLK = HALF // 128       # 16 query blocks
N_CORES = 8
DT = D // 128            # 4 contraction tiles
# xT column chunk boundaries (align with t-chunk needs + score windows)
XCH = [(0, 640), (640, 1152), (1152, 1664), (1664, 2304)]

_cached = {}


def _build_program():
    import concourse.bass as bass
    import concourse.tile as tile
    import concourse.mybir as mybir
    from concourse import bacc

    f32 = mybir.dt.float32
    f32r = mybir.dt.float32r
    bf16 = mybir.dt.bfloat16
    AF = mybir.ActivationFunctionType
    AX = mybir.AxisListType
    OP = mybir.AluOpType

    nc = bacc.Bacc("TRN2", target_bir_lowering=False, debug=False,
                   num_devices=N_CORES)

    id_d = nc.dram_tensor("ident", [128, 128], f32r, kind="ExternalInput").ap()
    A_d = nc.dram_tensor("A", [D, D], f32r, kind="ExternalInput").ap()
    wb_d = nc.dram_tensor("wb", [D, 1], f32, kind="ExternalInput").ap()
    xT_d = nc.dram_tensor("xT", [D, SK], f32r, kind="ExternalInput").ap()
    xr_d = nc.dram_tensor("xr", [SK, D], bf16, kind="ExternalInput").ap()
    mk_d = nc.dram_tensor("mk", [2, 128, WIN], bf16, kind="ExternalInput").ap()
    o_d = nc.dram_tensor("o", [HALF, D], f32, kind="ExternalOutput").ap()
    l_d = nc.dram_tensor("l", [128, NBLK], f32, kind="ExternalOutput").ap()

    with tile.TileContext(nc) as tc:
        with (
            tc.tile_pool(name="big", bufs=1) as big,
            tc.tile_pool(name="pp", bufs=2) as pp,
            tc.tile_pool(name="ppm", bufs=2) as ppm,
            tc.tile_pool(name="ppt", bufs=2) as ppt,
            tc.tile_pool(name="po", bufs=2) as po,
            tc.tile_pool(name="stat", bufs=4) as stat,
            tc.tile_pool(name="psPO", bufs=3, space="PSUM") as psPO,
            tc.tile_pool(name="psS", bufs=3, space="PSUM") as psS,
            tc.tile_pool(name="psT", bufs=2, space="PSUM") as psT,
        ):
            # ---- resident tiles ----
            mk = big.tile([128, 2, WIN], bf16, tag="mk", name="mk")
            wb = big.tile([128, DT], f32, tag="wb", name="wb")
            A_sb = big.tile([128, DT, D], f32r, tag="A", name="A")
            xT = big.tile([128, DT, SK], f32r, tag="xT", name="xT")
            xr = big.tile([128, SK // 128, D], bf16, tag="xr", name="xr")
            tT = big.tile([128, DT, HALF], f32r, tag="tT", name="tT")
            l_all = big.tile([128, NBLK], f32, tag="l", name="l_all")
            ident = big.tile([128, 128], f32r, tag="ident", name="ident")

            # ---- input DMAs, smallest/most-urgent first ----
            nc.sync.dma_start(ident, id_d)
            for v in range(2):
                nc.sync.dma_start(mk[:, v, :], mk_d[v])
            nc.sync.dma_start(
                wb[:, :], wb_d.rearrange("(t p) o -> p (t o)", t=DT))
            nc.sync.dma_start(
                A_sb[:, :, :], A_d.rearrange("(t p) d -> p t d", t=DT))

            def dma_xT_chunk(c):
                c0, c1 = XCH[c]
                for dt_i in range(DT):
                    nc.sync.dma_start(
                        xT[:, dt_i, c0:c1],
                        xT_d[dt_i * 128:(dt_i + 1) * 128, c0:c1])

            def dma_xr_piece(j0, j1):
                nc.sync.dma_start(
                    xr[:, j0:j1, :],
                    xr_d.rearrange("(j p) d -> p j d", j=SK // 128)[:, j0:j1, :])

            dma_xT_chunk(0)
            dma_xr_piece(0, 12)
            dma_xT_chunk(1)
            dma_xr_piece(12, 18)
            dma_xT_chunk(2)
            dma_xT_chunk(3)

            # ---- emitters ----
            def emit_tchunk(c):
                # tT[:, ht, 512c:512c+512] = (x @ A + wbeta).T chunk
                c0 = 512 * c
                for ht in range(DT):
                    ps = psPO.tile([128, 512], f32, tag="po")
                    for dt_i in range(DT):
                        nc.tensor.matmul(
                            ps,
                            lhsT=A_sb[:, dt_i, ht * 128:(ht + 1) * 128],
                            rhs=xT[:, dt_i, HALO + c0:HALO + c0 + 512],
                            start=(dt_i == 0),
                            stop=(dt_i == DT - 1),
                        )
                    if ht % 2 == 0:
                        nc.scalar.activation(
                            tT[:, ht, c0:c0 + 512], ps,
                            AF.Identity, bias=wb[:, ht:ht + 1], scale=1.0)
                    else:
                        nc.vector.tensor_scalar_add(
                            tT[:, ht, c0:c0 + 512], ps, wb[:, ht:ht + 1])

            def emit_scores(b):
                j0 = b * 128
                s_ps = psS.tile([128, WIN], f32, tag="s")
                for ht in range(DT):
                    nc.tensor.matmul(
                        s_ps,
                        lhsT=tT[:, ht, j0:j0 + 128],
                        rhs=xT[:, ht, j0:j0 + WIN],
                        start=(ht == 0),
                        stop=(ht == DT - 1),
                    )
                m = stat.tile([128, 1], f32, tag="m")
                nc.vector.reduce_max(m, s_ps, axis=AX.X)
                negm = stat.tile([128, 1], f32, tag="negm")
                nc.scalar.mul(negm, m, -1.0)
                p_sb = pp.tile([128, WIN], bf16, tag="p")
                nc.scalar.activation(p_sb, s_ps, AF.Exp,
                                     bias=negm, scale=1.0)
                pm_sb = ppm.tile([128, WIN], f32r, tag="pm")
                nc.vector.tensor_tensor(
                    pm_sb, p_sb, mk[:, 1 if b == 0 else 0, :], op=OP.mult)
                nc.vector.reduce_sum(l_all[:, b:b + 1], pm_sb, axis=AX.X)
                return pm_sb

            def emit_ta(b, pm_sb):
                pT_ps = psT.tile([128, 3, 128], f32r, tag="pT")
                for jt in range(3):
                    nc.tensor.transpose(
                        pT_ps[:, jt, :],
                        pm_sb[:, jt * 128:(jt + 1) * 128],
                        ident)
                pT_sb = ppt.tile([128, 3, 128], bf16, tag="pTs")
                nc.scalar.copy(pT_sb, pT_ps)
                o_ps = psPO.tile([128, 512], f32, tag="po")
                for jt in range(3):
                    nc.tensor.matmul(
                        o_ps,
                        lhsT=pT_sb[:, jt, :],
                        rhs=xr[:, b + jt, :],
                        start=(jt == 0),
                        stop=(jt == 2),
                    )
                o_sb = po.tile([128, 512], f32, tag="o")
                if b % 2 == 0:
                    nc.vector.tensor_copy(o_sb, o_ps)
                else:
                    nc.scalar.copy(o_sb, o_ps)
                nc.sync.dma_start(o_d[b * 128:(b + 1) * 128, :], o_sb)

            # ---- pipelined emission: t-chunks interleaved with blocks ----
            pms = {}
            emit_tchunk(0)
            pms[0] = emit_scores(0)
            pms[1] = emit_scores(1)
            emit_ta(0, pms.pop(0))
            pms[2] = emit_scores(2)
            emit_ta(1, pms.pop(1))
            pms[3] = emit_scores(3)
            emit_ta(2, pms.pop(2))
            for c in (1, 2, 3):
                emit_tchunk(c)
                for b in range(4 * c, 4 * c + 4):
                    pms[b] = emit_scores(b)
                    emit_ta(b - 1, pms.pop(b - 1))
            emit_ta(15, pms.pop(15))
            nc.sync.dma_start(l_d, l_all)

    nc.compile()
    return nc


def _get_program():
    if "nc" not in _cached:
        _cached["nc"] = _build_program()
    return _cached["nc"]


def _make_masks():
    # multiplicative band masks in the [query-row r, window-col c] frame:
    # valid iff 1 <= c - r <= 255; edge variant (block 0) also needs
    # c >= 128 (cols [0,128) are the zero-padded pre-sequence halo).
    r = np.arange(128)[:, None]
    c = np.arange(WIN)[None, :]
    band = (c - r >= 1) & (c - r <= 255)
    base = band.astype(np.float32)
    edge = (band & (c >= 128)).astype(np.float32)
    return np.stack([base, edge])


def kernel(x, Wq_w, Wq_b, Wk_w, Wk_b, _trace=False):
    import ml_dtypes
    from concourse.bass_utils import run_bass_kernel_spmd

    x = np.asarray(x, np.float32)
    Wq_w = np.asarray(Wq_w, np.float64)
    Wk_w = np.asarray(Wk_w, np.float64)
    Wq_b = np.asarray(Wq_b, np.float64)

    # fold both projections into one: t = x@A + wbeta, scores = t @ x^T
    A = np.ascontiguousarray((Wq_w.T @ Wk_w).astype(np.float32))
    wbeta = (Wk_w.T @ Wq_b).astype(np.float32).reshape(D, 1)
    masks = _make_masks().astype(ml_dtypes.bfloat16)

    nc = _get_program()

    in_maps = []
    for core in range(N_CORES):
        b, h = divmod(core, 2)
        x_halo = np.zeros((SK, D), np.float32)
        if h == 0:
            x_halo[HALO:] = x[b, 0:HALF + HALO]
        else:
            x_halo[HALO:] = x[b, S - HALF - HALO:][::-1]
        in_maps.append({
            "ident": np.eye(128, dtype=np.float32),
            "A": A,
            "wb": wbeta,
            "xT": np.ascontiguousarray(x_halo.T),
            "xr": x_halo.astype(ml_dtypes.bfloat16),
            "mk": masks,
        })

    res = run_bass_kernel_spmd(nc, in_maps, core_ids=list(range(N_CORES)),
                               trace=_trace)
    _cached["last_result"] = res

    y = np.zeros((B, S, D), np.float32)
    for core in range(N_CORES):
        b, h = divmod(core, 2)
        o = np.asarray(res.results[core]["o"], np.float64)
        l = np.asarray(res.results[core]["l"], np.float64)
        o = o / l.T.reshape(HALF, 1)
        if h == 0:
            y[b, :HALF] = o
        else:
            y[b, HALF:] = o[::-1]
    return y
